# revision 1
# baseline (speedup 1.0000x reference)
# Trainium2 Bass kernel for DenseFeatureNumericEmbedding.
#
# Math (per batch row b, feature f):
#   h[b,f,:]  = relu(x[b,f] * W1[f,:] + b1[f,:])          # Linear(1,H) + ReLU
#   emb[b,f,:] = W2[f] @ h[b,f,:] + b2[f,:]               # Linear(H,E)
#   out[b]    = concat_f emb[b,f,:]                       # [B, F*E]
#
# Shapes: B=16384, F=128, H=64, E=16.  8 NeuronCores, batch-sharded (2048 rows/core).
#
# Device pipeline per core (per 1024-row chunk, per feature-pair j = (2j, 2j+1)):
#   1. xT load: DMA-xbar transpose straight from DRAM (bf16, hi/lo column-
#      interleaved halves) -> xT tiles [feat-row, b] in SBUF.
#   2. L1 "broadcast" matmul: 0/1 selector stationary [K=128, M=128] x full xT tile
#      -> PSUM [128p = (2 feats x 64 h-slots), b] = exact fp32 x (hi+lo summed).
#   3. Fused drain at FD=1024 (alternating engines):
#        ACT:  h = relu(scale[p]*x + bias[p])             (per-partition W1/b1 columns)
#        DVE:  h = max(W1[p]*x, -b1[p]) = relu(W1 x + b1) - b1  (residual folded into b2)
#      -> h tiles [128, 1024] bf16 in SBUF.
#   4. L2 matmul: stationary block-diag W2 pair [K=128, M=32] bf16, tile_position
#      col-packed, 4 pairs x 2 halves -> PSUM out [128p = 8 feats x 16 e, b] fp32.
#   5. Drain + b2 bias (per-partition, fp32) -> SBUF [fe, b]; PE transpose 128x128
#      blocks; drain -> out_sb [b, fe] fp32; DMA contiguous row-blocks to DRAM,
#      shipped in column-slabs as groups complete so the tail DMA is short.
#
# OUT_BF16: route the output transpose through bf16 (2x DVE drain of the 16-bit
# transpose PSUM) at the cost of bf16-rounding the output.  Off: output is exact
# fp32 arithmetic downstream of the bf16 h/W2 matmul inputs.

import numpy as np
import ml_dtypes

BF16 = ml_dtypes.bfloat16

B, F, H, E = 16384, 128, 64, 16
NCORES = 8
BC = B // NCORES            # rows per core
CH = 1024                   # batch columns per chunk
FE = F * E                  # output width
NPAIR = F // 2              # feature pairs
NGROUP = F // 8             # groups of 8 features (one out-psum tile each)

OUT_BF16 = False


def _act_pair(j):
    """Pairs whose L1 drain runs on ScalarE (rest on VectorE). Keep in sync with
    the residual fold in _pack_weights."""
    return j % 2 == 0 or j in (1, 21, 41)


def _pack_weights(W1, b1, W2, b2):
    W1 = np.asarray(W1, np.float32)
    b1 = np.asarray(b1, np.float32)
    W2 = np.asarray(W2, np.float32)
    b2 = np.asarray(b2, np.float32)

    # Per-partition L1 scale/bias columns: partition p of pair j holds
    # (feature 2j + p//64, h = p%64).
    scl = np.zeros((128, NPAIR), np.float32)
    bia = np.zeros((128, NPAIR), np.float32)
    for j in range(NPAIR):
        scl[:64, j] = W1[2 * j]
        scl[64:, j] = W1[2 * j + 1]
        bia[:64, j] = b1[2 * j]
        bia[64:, j] = b1[2 * j + 1]

    # L2 stationaries: block-diag per pair, [K=128 (2x64 h), M=32 (2x16 e)].
    w2sb = np.zeros((128, NPAIR * 32), np.float32)
    for j in range(NPAIR):
        w2sb[:64, 32 * j : 32 * j + 16] = W2[2 * j].T          # [H, E]
        w2sb[64:, 32 * j + 16 : 32 * j + 32] = W2[2 * j + 1].T

    # DVE-drained pairs (odd j) produce h' = relu(.) - b1; fold the residual
    # sum_h W2[f,e,h]*b1[f,h] back into the output bias.
    resid = np.einsum("feh,fh->fe", W2, b1)
    b2adj = b2.copy()
    for f in range(F):
        if not _act_pair(f // 2):
            b2adj[f] += resid[f]

    # Output bias columns: partition p of group g = (q=p//32, d=(p%32)//16, e=p%16)
    # -> feature 8g + 2q + d.
    b2col = np.zeros((128, NGROUP), np.float32)
    for g in range(NGROUP):
        for q in range(4):
            for d in range(2):
                f = 8 * g + 2 * q + d
                lo = 32 * q + 16 * d
                b2col[lo : lo + 16, g] = b2adj[f]

    # L1 broadcast selector stationaries, one [K=128, M=128] 0/1 matrix per pair:
    # rows (4j)%128 + (0..3) are the (hi f0, lo f0, hi f1, lo f1) moving rows;
    # out col m<64 -> feat0 (rows 0,1), m>=64 -> feat1 (rows 2,3).
    selq = np.zeros((128, NPAIR * 128), np.float32)
    for j in range(NPAIR):
        p0 = (4 * j) % 128
        m0 = 128 * j
        selq[p0 + 0, m0 : m0 + 64] = 1.0
        selq[p0 + 1, m0 : m0 + 64] = 1.0
        selq[p0 + 2, m0 + 64 : m0 + 128] = 1.0
        selq[p0 + 3, m0 + 64 : m0 + 128] = 1.0

    ident = np.eye(128, dtype=np.float32)
    return dict(
        scl=scl,
        bia=bia,
        bianeg=-bia,
        w2sb=w2sb.astype(BF16),
        b2col=b2col,
        selqa=selq[:, : NPAIR * 64].astype(BF16),
        selqb=selq[:, NPAIR * 64 :].astype(BF16),
        identf=ident,
    )


def _prep_x(xs):
    """Split fp32 x into bf16 hi/lo, column-interleaved (col 2f = hi, 2f+1 = lo),
    shipped as two contiguous halves (features 0-63 / 64-127) so the xbar
    transpose DMA reads DRAM linearly."""
    xs = np.asarray(xs, np.float32)
    xh = xs.astype(BF16)
    xl = (xs - xh.astype(np.float32)).astype(BF16)
    x_il = np.empty((xs.shape[0], 2 * F), BF16)
    x_il[:, 0::2] = xh
    x_il[:, 1::2] = xl
    return (
        np.ascontiguousarray(x_il[:, 0:128]),
        np.ascontiguousarray(x_il[:, 128 : 2 * F]),
    )


def _build(nrows):
    from contextlib import ExitStack
    import concourse.bacc as bacc
    import concourse.mybir as mybir
    import concourse.tile as tile

    dt = mybir.dt
    AF = mybir.ActivationFunctionType
    ALU = mybir.AluOpType

    nchunk = nrows // CH
    nsub = CH // 128            # 128-row sub-blocks per chunk
    ot_dt = dt.bfloat16 if OUT_BF16 else dt.float32
    nc = bacc.Bacc(None, target_bir_lowering=False)

    x_ila_d = nc.declare_dram_parameter("x_ila", [nrows, F], dt.bfloat16, isOutput=False)
    x_ilb_d = nc.declare_dram_parameter("x_ilb", [nrows, F], dt.bfloat16, isOutput=False)
    scl_d = nc.declare_dram_parameter("scl", [128, NPAIR], dt.float32, isOutput=False)
    bia_d = nc.declare_dram_parameter("bia", [128, NPAIR], dt.float32, isOutput=False)
    bianeg_d = nc.declare_dram_parameter("bianeg", [128, NPAIR], dt.float32, isOutput=False)
    w2sb_d = nc.declare_dram_parameter("w2sb", [128, NPAIR * 32], dt.bfloat16, isOutput=False)
    b2col_d = nc.declare_dram_parameter("b2col", [128, NGROUP], dt.float32, isOutput=False)
    selqa_d = nc.declare_dram_parameter("selqa", [128, NPAIR * 64], dt.bfloat16, isOutput=False)
    selqb_d = nc.declare_dram_parameter("selqb", [128, NPAIR * 64], dt.bfloat16, isOutput=False)
    identf_d = nc.declare_dram_parameter("identf", [128, 128], dt.float32, isOutput=False)
    out_d = nc.declare_dram_parameter("out", [nrows, FE], dt.float32, isOutput=True)

    with tile.TileContext(nc) as tc, ExitStack() as ctx:
        const = ctx.enter_context(tc.tile_pool(name="const", bufs=1))
        xt_p = ctx.enter_context(tc.tile_pool(name="xt", bufs=2))
        h_p = ctx.enter_context(tc.tile_pool(name="h", bufs=10))
        ot_p = ctx.enter_context(tc.tile_pool(name="ot", bufs=4))
        outsb_p = ctx.enter_context(tc.tile_pool(name="outsb", bufs=2))
        # PSUM budget (8 banks): ps_x 2x[128,1024]f32 = 4, ps_o 2x[128,512]f32 = 2,
        # ps_t2 1x[128,512] = 1, ps_xt 1x[128,1024]bf16 = 1.
        ps_x = ctx.enter_context(tc.tile_pool(name="ps_x", bufs=2, space="PSUM"))
        ps_o = ctx.enter_context(tc.tile_pool(name="ps_o", bufs=1, space="PSUM"))
        ps_t2 = ctx.enter_context(tc.tile_pool(name="ps_t2", bufs=1, space="PSUM"))

        sclT = const.tile([128, NPAIR], dt.float32, tag="scl")
        biaT = const.tile([128, NPAIR], dt.float32, tag="bia")
        bianegT = const.tile([128, NPAIR], dt.float32, tag="bianeg")
        w2T = const.tile([128, NPAIR * 32], dt.bfloat16, tag="w2")
        b2colT = const.tile([128, NGROUP], dt.float32, tag="b2col")
        selqaT = const.tile([128, NPAIR * 64], dt.bfloat16, tag="selqa")
        selqbT = const.tile([128, NPAIR * 64], dt.bfloat16, tag="selqb")
        identfT = const.tile([128, 128], dt.float32, tag="identf")
        nc.sync.dma_start(selqaT[:], selqa_d[:])
        nc.sync.dma_start(sclT[:], scl_d[:])
        nc.sync.dma_start(biaT[:], bia_d[:])
        nc.sync.dma_start(bianegT[:], bianeg_d[:])

        identoT = identfT

        # Transpose x straight from DRAM into SBUF via the DMA xbar (bf16),
        # prefetching both chunks up front on the ACT DGE ring.
        xts = []
        for c in range(nchunk):
            xta = xt_p.tile([128, CH], dt.bfloat16, tag="xta")
            xtb = xt_p.tile([128, CH], dt.bfloat16, tag="xtb")
            for hh in range(2):
                r0, r1 = c * CH + hh * (CH // 2), c * CH + (hh + 1) * (CH // 2)
                c0, c1 = hh * (CH // 2), (hh + 1) * (CH // 2)
                nc.scalar.dma_start_transpose(xta[:, c0:c1], x_ila_d[r0:r1, :])
                nc.scalar.dma_start_transpose(xtb[:, c0:c1], x_ilb_d[r0:r1, :])
            xts.append((xta, xtb))
            if c == 0:
                # consts not needed until the first out_phase: keep them off the
                # startup critical path (xbar transposes serialize with copies)
                nc.sync.dma_start(selqbT[:], selqb_d[:])
                nc.sync.dma_start(w2T[:], w2sb_d[:])
                nc.sync.dma_start(b2colT[:], b2col_d[:])
                nc.sync.dma_start(identfT[:], identf_d[:])

        for c in range(nchunk):
            xta, xtb = xts[c]

            out_sb = outsb_p.tile([128, nsub, NGROUP, 128], dt.float32, tag="out_sb")

            def out_l2(g, hts):
                # L2 matmuls (q-outer / half-inner: one stationary load serves
                # both 512-halves) + bias drain to [fe, b] SBUF tiles.
                po = ps_o.tile([128, 2, 512], dt.float32, tag="ps_out")
                for q in range(4):
                    j = 4 * g + q
                    for half in range(2):
                        nc.tensor.matmul(
                            po[32 * q : 32 * q + 32, half, :],
                            w2T[:, 32 * j : 32 * j + 32],
                            hts[q][:, 512 * half : 512 * (half + 1)],
                            start=True,
                            stop=True,
                            tile_position=(0, 32 * q),
                        )
                # single FD=1024 bias drain over both halves
                ot = ot_p.tile([128, 2, 512], ot_dt, tag="ot")
                nc.vector.tensor_scalar(
                    ot[:], po[:], b2colT[:, g : g + 1], None, ALU.add
                )
                return ot

            def out_tr(g, ot):
                # PE transpose to [b, fe] + one FD=1024 drain into out_sb (a full
                # pipeline stage behind out_l2 so the bias drain is long done).
                pst = ps_t2.tile([128, 8, 128], ot_dt, tag="ps_ot")
                for t8 in range(8):
                    nc.tensor.transpose(
                        pst[:, t8, :],
                        ot[:, t8 // 4, 128 * (t8 % 4) : 128 * (t8 % 4 + 1)],
                        identoT[:],
                    )
                dst = out_sb[:, :, g, :]
                nc.scalar.copy(dst, pst[:])

            def ship(g):
                # column-slab DMAs as soon as a slab's last group is in out_sb
                bounds = {NGROUP // 2 - 1: (0, NGROUP // 2),
                          3 * NGROUP // 4 - 1: (NGROUP // 2, 3 * NGROUP // 4),
                          7 * NGROUP // 8 - 1: (3 * NGROUP // 4, 7 * NGROUP // 8),
                          NGROUP - 1: (7 * NGROUP // 8, NGROUP)}
                if g in bounds:
                    lo, hi = bounds[g]
                    for t in range(nsub):
                        r0 = c * CH + t * 128
                        nc.sync.dma_start(
                            out_d[r0 : r0 + 128, 128 * lo : 128 * hi],
                            out_sb[:, t, lo:hi, :],
                        )

            pend_l2 = None
            pend_tr = None
            for g in range(NGROUP):
                hts = []
                for q in range(4):
                    j = 4 * g + q
                    xt = xta if j < 32 else xtb
                    ps = ps_x.tile([128, CH], dt.float32, tag="ps_x")
                    selt = selqaT if j < 32 else selqbT
                    sel = selt[:, 128 * (j % 32) : 128 * (j % 32 + 1)]
                    nc.tensor.matmul(
                        ps[:, 0:512], sel, xt[:, 0:512], start=True, stop=True
                    )
                    nc.tensor.matmul(
                        ps[:, 512:1024], sel, xt[:, 512:1024], start=True, stop=True
                    )
                    ht = h_p.tile([128, CH], dt.bfloat16, tag="h")
                    if _act_pair(j):
                        nc.scalar.activation(
                            ht[:],
                            ps[:],
                            AF.Relu,
                            bias=biaT[:, j : j + 1],
                            scale=sclT[:, j : j + 1],
                        )
                    else:
                        nc.vector.tensor_scalar(
                            ht[:],
                            ps[:],
                            sclT[:, j : j + 1],
                            bianegT[:, j : j + 1],
                            ALU.mult,
                            ALU.max,
                        )
                    hts.append(ht)
                if pend_l2 is not None:
                    gl, hl = pend_l2
                    ots = out_l2(gl, hl)
                    if pend_tr is not None:
                        gt, ot_prev = pend_tr
                        out_tr(gt, ot_prev)
                        ship(gt)
                    pend_tr = (gl, ots)
                pend_l2 = (g, hts)
            # epilogue: flush both pipeline stages
            gl, hl = pend_l2
            ots = out_l2(gl, hl)
            if pend_tr is not None:
                gt, ot_prev = pend_tr
                out_tr(gt, ot_prev)
                ship(gt)
            out_tr(gl, ots)
            ship(gl)

    nc.compile()
    return nc


_NC_CACHE = {}


def _get_program(nrows):
    if nrows not in _NC_CACHE:
        _NC_CACHE[nrows] = _build(nrows)
    return _NC_CACHE[nrows]


def kernel(x, W1, b1, W2, b2, _trace=False):
    from concourse.bass_utils import run_bass_kernel_spmd

    x = np.asarray(x, np.float32)
    cfg = _pack_weights(W1, b1, W2, b2)
    nc = _get_program(BC)
    wkeys = ("scl", "bia", "bianeg", "w2sb", "b2col", "selqa", "selqb", "identf")
    in_maps = []
    for c in range(NCORES):
        xa, xb = _prep_x(x[c * BC : (c + 1) * BC])
        m = {"x_ila": xa, "x_ilb": xb}
        for k in wkeys:
            m[k] = cfg[k]
        in_maps.append(m)
    res = run_bass_kernel_spmd(
        nc, in_maps, core_ids=list(range(NCORES)), trace=_trace
    )
    out = np.concatenate([r["out"] for r in res.results], axis=0)
    if _trace:
        kernel.last_result = res
    return np.ascontiguousarray(out.astype(np.float32))



# revision 2
# speedup vs baseline: 1.0805x; 1.0805x over previous
# Trainium2 Bass kernel for DenseFeatureNumericEmbedding.
#
# Math (per batch row b, feature f):
#   h[b,f,:]  = relu(x[b,f] * W1[f,:] + b1[f,:])          # Linear(1,H) + ReLU
#   emb[b,f,:] = W2[f] @ h[b,f,:] + b2[f,:]               # Linear(H,E)
#   out[b]    = concat_f emb[b,f,:]                       # [B, F*E]
#
# Shapes: B=16384, F=128, H=64, E=16.  8 NeuronCores, batch-sharded (2048 rows/core).
#
# Device pipeline per core (per 1024-col chunk, per feature pair j):
#   1. xT load: x split into fp8e4 hi/lo residue pair on host, consecutive batch
#      rows byte-packed into uint16 so the DMA-xbar transpose (2-byte only)
#      lands pure hi / lo fp8 slabs in SBUF: xt [128 feat, 2 (hi,lo), CH].
#   2. L1 broadcast matmul in fp8 DoubleRow perf mode: 0/1 selector stationary
#      [K=128, 2, M=128] x xt slab pair -> psum[p, b] = x_hi + x_lo = x exact to
#      ~2^-9, at 2 moving cols/cycle (half the bf16 cost).
#   3. Fused drain at FD=1024 (alternating engines):
#        ACT:  h = relu(scale[p]*x + bias[p])             (per-partition W1/b1)
#        DVE:  h = max(W1[p]*x, -b1[p]) = relu(W1 x + b1) - b1  (resid -> b2)
#      -> h tiles [128, 1024] fp16 in SBUF.
#   4. L2 matmul: stationary block-diag W2 pair [K=128, M=32] fp16,
#      tile_position col-packed, 4 pairs x 2 halves -> psum [128 = 8f x 16e, b].
#   5. DVE drain + b2 bias -> out_sb [fe, b] fp16; DMA straight to DRAM in
#      [FE, BC] layout.  The host unshard transposes each core's [FE, BC] block
#      to [BC, FE] while upcasting to fp32 (no PE transposes on device).

import numpy as np
import ml_dtypes

BF16 = ml_dtypes.bfloat16
FP8 = ml_dtypes.float8_e4m3
FP16 = np.float16

B, F, H, E = 16384, 128, 64, 16
NCORES = 8
BC = B // NCORES            # rows per core
CH = 1024                   # batch columns per chunk
FE = F * E                  # output width
NPAIR = F // 2              # feature pairs
NGROUP = F // 8             # groups of 8 features (one out-psum tile each)

USE_DR = True               # fp8 DoubleRow L1 (else bf16 hi/lo as fallback)


def _act_pair(j):
    """Pairs whose L1 drain runs on ScalarE (rest on VectorE).  Keep in sync
    with the residual fold in _pack_weights.  43:21 per chunk balances
    ACT(1.04us/op) vs DVE(1.19us/op + 16 L2 drains)."""
    return j % 3 != 2


def _pack_weights(W1, b1, W2, b2):
    W1 = np.asarray(W1, np.float32)
    b1 = np.asarray(b1, np.float32)
    W2 = np.asarray(W2, np.float32)
    b2 = np.asarray(b2, np.float32)

    # Per-partition L1 scale/bias columns: partition p of pair j holds
    # (feature 2j + p//64, h = p%64).
    scl = np.zeros((128, NPAIR), np.float32)
    bia = np.zeros((128, NPAIR), np.float32)
    for j in range(NPAIR):
        scl[:64, j] = W1[2 * j]
        scl[64:, j] = W1[2 * j + 1]
        bia[:64, j] = b1[2 * j]
        bia[64:, j] = b1[2 * j + 1]

    # L2 stationaries: block-diag per pair, [K=128 (2x64 h), M=32 (2x16 e)].
    w2sb = np.zeros((128, NPAIR * 32), np.float32)
    for j in range(NPAIR):
        w2sb[:64, 32 * j : 32 * j + 16] = W2[2 * j].T          # [H, E]
        w2sb[64:, 32 * j + 16 : 32 * j + 32] = W2[2 * j + 1].T

    # DVE-drained pairs produce h' = relu(.) - b1; fold the residual
    # sum_h W2[f,e,h]*b1[f,h] back into the output bias.
    resid = np.einsum("feh,fh->fe", W2, b1)
    b2adj = b2.copy()
    for f in range(F):
        if not _act_pair(f // 2):
            b2adj[f] += resid[f]

    # Output bias columns: psum partition p of group g = fe 128g + p.
    b2col = np.zeros((128, NGROUP), np.float32)
    for g in range(NGROUP):
        for q in range(4):
            for d in range(2):
                f = 8 * g + 2 * q + d
                lo = 32 * q + 16 * d
                b2col[lo : lo + 16, g] = b2adj[f]

    cfg = dict(
        scl=scl,
        bia=bia,
        bianeg=-bia,
        w2sb=w2sb.astype(FP16),
        b2col=b2col,
    )

    if USE_DR:
        # DoubleRow selector: sel3[p, s, m] per pair j -> partitions 2j (m<64)
        # and 2j+1 (m>=64) pass both hi (s=0) and lo (s=1) slabs.
        sel3 = np.zeros((128, 2, NPAIR * 128), np.float32)
        for j in range(NPAIR):
            m0 = 128 * j
            sel3[2 * j, :, m0 : m0 + 64] = 1.0
            sel3[2 * j + 1, :, m0 + 64 : m0 + 128] = 1.0
        cfg["sel3"] = np.ascontiguousarray(sel3).astype(FP8)
    else:
        # bf16 hi/lo fallback: 4 moving rows per pair (hi f0, lo f0, hi f1,
        # lo f1), column-interleaved halves as in the original kernel.
        selq = np.zeros((128, NPAIR * 128), np.float32)
        for j in range(NPAIR):
            p0 = (4 * j) % 128
            m0 = 128 * j
            selq[p0 + 0, m0 : m0 + 64] = 1.0
            selq[p0 + 1, m0 : m0 + 64] = 1.0
            selq[p0 + 2, m0 + 64 : m0 + 128] = 1.0
            selq[p0 + 3, m0 + 64 : m0 + 128] = 1.0
        cfg["selqa"] = selq[:, : NPAIR * 64].astype(BF16)
        cfg["selqb"] = selq[:, NPAIR * 64 :].astype(BF16)
    return cfg


def _prep_x_dr(xs):
    """fp8 hi/lo split, consecutive batch rows byte-packed into uint16 so the
    2-byte xbar transpose delivers pure fp8 slabs [feat, b]."""
    xs = np.asarray(xs, np.float32)
    xh = xs.astype(FP8)
    xl = (xs - xh.astype(np.float32)).astype(FP8)

    def pack(a):
        bts = a.view(np.uint8)
        return np.ascontiguousarray(
            bts[0::2, :].astype(np.uint16) | (bts[1::2, :].astype(np.uint16) << 8)
        )

    return pack(xh), pack(xl)


def _prep_x_bf(xs):
    xs = np.asarray(xs, np.float32)
    xh = xs.astype(BF16)
    xl = (xs - xh.astype(np.float32)).astype(BF16)
    x_il = np.empty((xs.shape[0], 2 * F), BF16)
    x_il[:, 0::2] = xh
    x_il[:, 1::2] = xl
    return (
        np.ascontiguousarray(x_il[:, 0:128]),
        np.ascontiguousarray(x_il[:, 128 : 2 * F]),
    )


def _build(nrows):
    from contextlib import ExitStack
    import concourse.bacc as bacc
    import concourse.mybir as mybir
    import concourse.tile as tile

    dt = mybir.dt
    AF = mybir.ActivationFunctionType
    ALU = mybir.AluOpType
    PM = mybir.MatmulPerfMode

    nchunk = nrows // CH
    nc = bacc.Bacc(None, target_bir_lowering=False)

    if USE_DR:
        xh_d = nc.declare_dram_parameter("xh", [nrows // 2, F], dt.uint16, isOutput=False)
        xl_d = nc.declare_dram_parameter("xl", [nrows // 2, F], dt.uint16, isOutput=False)
        sel3_d = nc.declare_dram_parameter(
            "sel3", [128, 2, NPAIR * 128], dt.float8e4, isOutput=False
        )
    else:
        x_ila_d = nc.declare_dram_parameter("x_ila", [nrows, F], dt.bfloat16, isOutput=False)
        x_ilb_d = nc.declare_dram_parameter("x_ilb", [nrows, F], dt.bfloat16, isOutput=False)
        selqa_d = nc.declare_dram_parameter("selqa", [128, NPAIR * 64], dt.bfloat16, isOutput=False)
        selqb_d = nc.declare_dram_parameter("selqb", [128, NPAIR * 64], dt.bfloat16, isOutput=False)
    scl_d = nc.declare_dram_parameter("scl", [128, NPAIR], dt.float32, isOutput=False)
    bia_d = nc.declare_dram_parameter("bia", [128, NPAIR], dt.float32, isOutput=False)
    bianeg_d = nc.declare_dram_parameter("bianeg", [128, NPAIR], dt.float32, isOutput=False)
    w2sb_d = nc.declare_dram_parameter("w2sb", [128, NPAIR * 32], dt.float16, isOutput=False)
    b2col_d = nc.declare_dram_parameter("b2col", [128, NGROUP], dt.float32, isOutput=False)
    out_d = nc.declare_dram_parameter("out", [FE, nrows], dt.float16, isOutput=True)

    with tile.TileContext(nc) as tc, ExitStack() as ctx:
        const = ctx.enter_context(tc.tile_pool(name="const", bufs=1))
        xt_p = ctx.enter_context(tc.tile_pool(name="xt", bufs=2))
        h_p = ctx.enter_context(tc.tile_pool(name="h", bufs=10))
        outsb_p = ctx.enter_context(tc.tile_pool(name="outsb", bufs=4))
        # PSUM budget (8 banks): ps_x 2x[128,1024]f32 = 4, ps_o 2x[128,2,512] = 4.
        ps_x = ctx.enter_context(tc.tile_pool(name="ps_x", bufs=2, space="PSUM"))
        ps_o = ctx.enter_context(tc.tile_pool(name="ps_o", bufs=2, space="PSUM"))

        sclT = const.tile([128, NPAIR], dt.float32, tag="scl")
        biaT = const.tile([128, NPAIR], dt.float32, tag="bia")
        bianegT = const.tile([128, NPAIR], dt.float32, tag="bianeg")
        w2T = const.tile([128, NPAIR * 32], dt.float16, tag="w2")
        b2colT = const.tile([128, NGROUP], dt.float32, tag="b2col")
        if USE_DR:
            sel3T = const.tile([128, 2, NPAIR * 128], dt.float8e4, tag="sel3")
            # Split the 2 MiB selector load across 4 DMAs so the head of the
            # pair loop isn't gated on the whole tile.
            qs = NPAIR * 128 // 2
            for s in range(2):
                nc.sync.dma_start(sel3T[:, s, 0:qs], sel3_d[:, s, 0:qs])
                nc.sync.dma_start(sel3T[:, s, qs:], sel3_d[:, s, qs:])
        else:
            selqaT = const.tile([128, NPAIR * 64], dt.bfloat16, tag="selqa")
            selqbT = const.tile([128, NPAIR * 64], dt.bfloat16, tag="selqb")
            nc.sync.dma_start(selqaT[:], selqa_d[:])
        nc.sync.dma_start(sclT[:], scl_d[:])
        nc.sync.dma_start(biaT[:], bia_d[:])
        nc.sync.dma_start(bianegT[:], bianeg_d[:])

        # Transpose x straight from DRAM into SBUF via the DMA xbar.
        xts = []
        for c in range(nchunk):
            if USE_DR:
                xt = xt_p.tile([128, 2, CH], dt.float8e4, tag="xt")
                r0, r1 = c * (CH // 2), (c + 1) * (CH // 2)
                nc.scalar.dma_start_transpose(
                    xt[:, 0, :].bitcast(dt.uint16), xh_d[r0:r1, :]
                )
                nc.scalar.dma_start_transpose(
                    xt[:, 1, :].bitcast(dt.uint16), xl_d[r0:r1, :]
                )
                xts.append(xt)
            else:
                xta = xt_p.tile([128, CH], dt.bfloat16, tag="xta")
                xtb = xt_p.tile([128, CH], dt.bfloat16, tag="xtb")
                for hh in range(2):
                    r0, r1 = c * CH + hh * (CH // 2), c * CH + (hh + 1) * (CH // 2)
                    c0, c1 = hh * (CH // 2), (hh + 1) * (CH // 2)
                    nc.scalar.dma_start_transpose(xta[:, c0:c1], x_ila_d[r0:r1, :])
                    nc.scalar.dma_start_transpose(xtb[:, c0:c1], x_ilb_d[r0:r1, :])
                xts.append((xta, xtb))
            if c == 0:
                if not USE_DR:
                    nc.sync.dma_start(selqbT[:], selqb_d[:])
                nc.sync.dma_start(w2T[:], w2sb_d[:])
                nc.sync.dma_start(b2colT[:], b2col_d[:])

        for c in range(nchunk):
            def out_l2(g, hts):
                # L2 matmuls + bias drain straight into [fe, b] layout.
                po = ps_o.tile([128, 2, 512], dt.float32, tag="ps_out")
                for q in range(4):
                    j = 4 * g + q
                    for half in range(2):
                        nc.tensor.matmul(
                            po[32 * q : 32 * q + 32, half, :],
                            w2T[:, 32 * j : 32 * j + 32],
                            hts[q][:, 512 * half : 512 * (half + 1)],
                            start=True,
                            stop=True,
                            tile_position=(0, 32 * q),
                        )
                ot = outsb_p.tile([128, CH], dt.float16, tag="ot")
                nc.vector.tensor_scalar(
                    ot[:], po[:], b2colT[:, g : g + 1], None, ALU.add
                )
                nc.sync.dma_start(
                    out_d[128 * g : 128 * (g + 1), c * CH : (c + 1) * CH], ot[:]
                )

            pend_l2 = None
            for g in range(NGROUP):
                hts = []
                for q in range(4):
                    j = 4 * g + q
                    ps = ps_x.tile([128, CH], dt.float32, tag="ps_x")
                    if USE_DR:
                        xt = xts[c]
                        sel = sel3T[:, :, 128 * j : 128 * (j + 1)]
                        for half in range(2):
                            nc.tensor.matmul(
                                ps[:, 512 * half : 512 * (half + 1)],
                                sel,
                                xt[:, :, 512 * half : 512 * (half + 1)],
                                start=True,
                                stop=True,
                                perf_mode=PM.DoubleRow,
                            )
                    else:
                        xta, xtb = xts[c]
                        xt = xta if j < 32 else xtb
                        selt = selqaT if j < 32 else selqbT
                        sel = selt[:, 128 * (j % 32) : 128 * (j % 32 + 1)]
                        nc.tensor.matmul(
                            ps[:, 0:512], sel, xt[:, 0:512], start=True, stop=True
                        )
                        nc.tensor.matmul(
                            ps[:, 512:1024], sel, xt[:, 512:1024], start=True, stop=True
                        )
                    ht = h_p.tile([128, CH], dt.float16, tag="h")
                    if _act_pair(j):
                        nc.scalar.activation(
                            ht[:],
                            ps[:],
                            AF.Relu,
                            bias=biaT[:, j : j + 1],
                            scale=sclT[:, j : j + 1],
                        )
                    else:
                        nc.vector.tensor_scalar(
                            ht[:],
                            ps[:],
                            sclT[:, j : j + 1],
                            bianegT[:, j : j + 1],
                            ALU.mult,
                            ALU.max,
                        )
                    hts.append(ht)
                if pend_l2 is not None:
                    out_l2(*pend_l2)
                pend_l2 = (g, hts)
            out_l2(*pend_l2)

    nc.compile()
    return nc


_NC_CACHE = {}


def _get_program(nrows):
    if nrows not in _NC_CACHE:
        _NC_CACHE[nrows] = _build(nrows)
    return _NC_CACHE[nrows]


def kernel(x, W1, b1, W2, b2, _trace=False):
    from concourse.bass_utils import run_bass_kernel_spmd

    x = np.asarray(x, np.float32)
    cfg = _pack_weights(W1, b1, W2, b2)
    nc = _get_program(BC)
    wkeys = (
        ("scl", "bia", "bianeg", "w2sb", "b2col", "sel3")
        if USE_DR
        else ("scl", "bia", "bianeg", "w2sb", "b2col", "selqa", "selqb")
    )
    in_maps = []
    for c in range(NCORES):
        xs = x[c * BC : (c + 1) * BC]
        if USE_DR:
            xh, xl = _prep_x_dr(xs)
            m = {"xh": xh, "xl": xl}
        else:
            xa, xb = _prep_x_bf(xs)
            m = {"x_ila": xa, "x_ilb": xb}
        for k in wkeys:
            m[k] = cfg[k]
        in_maps.append(m)
    res = run_bass_kernel_spmd(
        nc, in_maps, core_ids=list(range(NCORES)), trace=_trace
    )
    out = np.empty((B, FE), np.float32)
    for c in range(NCORES):
        oc = np.asarray(res.results[c]["out"])          # [FE, BC] fp16
        out[c * BC : (c + 1) * BC] = oc.T.astype(np.float32)
    if _trace:
        kernel.last_result = res
    return out


# revision 4
# speedup vs baseline: 1.0858x; 1.0049x over previous
# Trainium2 Bass kernel for DenseFeatureNumericEmbedding.
#
# Math (per batch row b, feature f):
#   h[b,f,:]  = relu(x[b,f] * W1[f,:] + b1[f,:])          # Linear(1,H) + ReLU
#   emb[b,f,:] = W2[f] @ h[b,f,:] + b2[f,:]               # Linear(H,E)
#   out[b]    = concat_f emb[b,f,:]                       # [B, F*E]
#
# Shapes: B=16384, F=128, H=64, E=16.  8 NeuronCores, batch-sharded (2048 rows/core).
#
# Device pipeline per core (per 1024-col chunk, per feature pair j):
#   1. xT load: DMA-xbar transpose straight from DRAM (bf16 hi/lo residue
#      rows, strip-permuted columns) -> xt tiles [row, b] in SBUF.
#   2. L1 broadcast matmul with K=32 ROW-STRIP stationaries: pair j's 0/1
#      selector lives in row strip 32*(j%4) (tile_position row), so
#      consecutive pairs hit different PE row-groups -> LDWEIGHTS is pulled
#      ahead of in-flight matmuls and row-tiled matmuls overlap.
#      psum [128p = 2 feats x 64 h-slots, b] = exact x (hi+lo summed).
#   3. Fused drain at FD=1024 (alternating engines):
#        ACT:  h = relu(scale[p]*x + bias[p])             (per-partition W1/b1)
#        DVE:  h = max(W1[p]*x, -b1[p]) = relu(W1 x + b1) - b1  (resid -> b2)
#      -> h tiles [128, 1024] fp16 in SBUF.
#   4. L2 matmul: stationary block-diag W2 pair [K=128, M=32] fp16,
#      tile_position col-packed, 4 pairs x 2 halves -> psum [128 = 8f x 16e, b].
#   5. DVE drain + b2 bias -> out_sb [fe, b] fp16; DMA straight to DRAM in
#      [FE, BC] layout.  The host unshard transposes each core's [FE, BC]
#      block to [BC, FE] while upcasting to fp32 (no PE transposes on device).
#   Warmup matmuls on the first-loaded const keep the PE HAM clock-gate busy
#   through its 3.4us activity window while the x transposes are in flight.

import numpy as np
import ml_dtypes

BF16 = ml_dtypes.bfloat16
FP16 = np.float16

B, F, H, E = 16384, 128, 64, 16
NCORES = 8
BC = B // NCORES            # rows per core
CH = 1024                   # batch columns per chunk
FE = F * E                  # output width
NPAIR = F // 2              # feature pairs
NGROUP = F // 8             # groups of 8 features (one out-psum tile each)
NWARM = 24                  # HAM warmup matmuls


def _strip_row(j):
    """xt row (and selector strip) of pair j's 4 hi/lo rows: consecutive pairs
    rotate through the 4 PE row strips."""
    s = j % 4
    u = (j % 32) // 4
    return 32 * s + 4 * u


def _act_pair(j):
    """Pairs whose L1 drain runs on ScalarE (rest on VectorE).  Keep in sync
    with the residual fold in _pack_weights.  43:21 per chunk balances
    ACT(1.04us/op) vs DVE(1.19us/op + 16 L2 drains)."""
    return j % 3 != 2


def _pack_weights(W1, b1, W2, b2):
    W1 = np.asarray(W1, np.float32)
    b1 = np.asarray(b1, np.float32)
    W2 = np.asarray(W2, np.float32)
    b2 = np.asarray(b2, np.float32)

    # Per-partition L1 scale/bias columns: partition p of pair j holds
    # (feature 2j + p//64, h = p%64).
    scl = np.zeros((128, NPAIR), np.float32)
    bia = np.zeros((128, NPAIR), np.float32)
    for j in range(NPAIR):
        scl[:64, j] = W1[2 * j]
        scl[64:, j] = W1[2 * j + 1]
        bia[:64, j] = b1[2 * j]
        bia[64:, j] = b1[2 * j + 1]

    # L2 stationaries: block-diag per pair, [K=128 (2x64 h), M=32 (2x16 e)].
    w2sb = np.zeros((128, NPAIR * 32), np.float32)
    for j in range(NPAIR):
        w2sb[:64, 32 * j : 32 * j + 16] = W2[2 * j].T          # [H, E]
        w2sb[64:, 32 * j + 16 : 32 * j + 32] = W2[2 * j + 1].T

    # DVE-drained pairs produce h' = relu(.) - b1; fold the residual
    # sum_h W2[f,e,h]*b1[f,h] back into the output bias.
    resid = np.einsum("feh,fh->fe", W2, b1)
    b2adj = b2.copy()
    for f in range(F):
        if not _act_pair(f // 2):
            b2adj[f] += resid[f]

    # Output bias columns: psum partition p of group g = fe 128g + p.
    b2col = np.zeros((128, NGROUP), np.float32)
    for g in range(NGROUP):
        for q in range(4):
            for d in range(2):
                f = 8 * g + 2 * q + d
                lo = 32 * q + 16 * d
                b2col[lo : lo + 16, g] = b2adj[f]

    # L1 selector: pair j's 4 moving rows (hi f0, lo f0, hi f1, lo f1) live at
    # xt rows _strip_row(j)..+4 of its half-tile; out col m<64 -> f0, else f1.
    sels = np.zeros((128, NPAIR * 128), np.float32)
    for j in range(NPAIR):
        r0 = _strip_row(j)
        m0 = 128 * j
        sels[r0 + 0, m0 : m0 + 64] = 1.0
        sels[r0 + 1, m0 : m0 + 64] = 1.0
        sels[r0 + 2, m0 + 64 : m0 + 128] = 1.0
        sels[r0 + 3, m0 + 64 : m0 + 128] = 1.0

    return dict(
        scl=scl,
        bia=bia,
        bianeg=-bia,
        w2sb=w2sb.astype(FP16),
        b2col=b2col,
        sels=sels.astype(BF16),
    )


def _prep_x(xs):
    """Split fp32 x into bf16 hi/lo rows, strip-permuted so pair j's rows sit
    at _strip_row(j) of its half-tile, shipped as two 128-col halves."""
    xs = np.asarray(xs, np.float32)
    xh = xs.astype(BF16)
    xl = (xs - xh.astype(np.float32)).astype(BF16)
    xa = np.empty((xs.shape[0], 128), BF16)
    xb = np.empty((xs.shape[0], 128), BF16)
    for j in range(NPAIR):
        dst = xa if j < 32 else xb
        r0 = _strip_row(j)
        dst[:, r0 + 0] = xh[:, 2 * j]
        dst[:, r0 + 1] = xl[:, 2 * j]
        dst[:, r0 + 2] = xh[:, 2 * j + 1]
        dst[:, r0 + 3] = xl[:, 2 * j + 1]
    return np.ascontiguousarray(xa), np.ascontiguousarray(xb)


def _build(nrows):
    from contextlib import ExitStack
    import concourse.bacc as bacc
    import concourse.mybir as mybir
    import concourse.tile as tile

    dt = mybir.dt
    AF = mybir.ActivationFunctionType
    ALU = mybir.AluOpType

    nchunk = nrows // CH
    nc = bacc.Bacc(None, target_bir_lowering=False)

    x_ila_d = nc.declare_dram_parameter("x_ila", [nrows, F], dt.bfloat16, isOutput=False)
    x_ilb_d = nc.declare_dram_parameter("x_ilb", [nrows, F], dt.bfloat16, isOutput=False)
    sels_d = nc.declare_dram_parameter("sels", [128, NPAIR * 128], dt.bfloat16, isOutput=False)
    scl_d = nc.declare_dram_parameter("scl", [128, NPAIR], dt.float32, isOutput=False)
    bia_d = nc.declare_dram_parameter("bia", [128, NPAIR], dt.float32, isOutput=False)
    bianeg_d = nc.declare_dram_parameter("bianeg", [128, NPAIR], dt.float32, isOutput=False)
    w2sb_d = nc.declare_dram_parameter("w2sb", [128, NPAIR * 32], dt.float16, isOutput=False)
    b2col_d = nc.declare_dram_parameter("b2col", [128, NGROUP], dt.float32, isOutput=False)
    out_d = nc.declare_dram_parameter("out", [FE, nrows], dt.float16, isOutput=True)

    with tile.TileContext(nc) as tc, ExitStack() as ctx:
        const = ctx.enter_context(tc.tile_pool(name="const", bufs=1))
        xt_p = ctx.enter_context(tc.tile_pool(name="xt", bufs=2))
        h_p = ctx.enter_context(tc.tile_pool(name="h", bufs=10))
        outsb_p = ctx.enter_context(tc.tile_pool(name="outsb", bufs=4))
        # PSUM budget (8 banks): ps_x 3x[128,1024]f32 = 6, ps_o 1x[128,2,512] = 2.
        ps_x = ctx.enter_context(tc.tile_pool(name="ps_x", bufs=3, space="PSUM"))
        ps_o = ctx.enter_context(tc.tile_pool(name="ps_o", bufs=1, space="PSUM"))

        sclT = const.tile([128, NPAIR], dt.float32, tag="scl")
        biaT = const.tile([128, NPAIR], dt.float32, tag="bia")
        bianegT = const.tile([128, NPAIR], dt.float32, tag="bianeg")
        w2T = const.tile([128, NPAIR * 32], dt.float16, tag="w2")
        b2colT = const.tile([128, NGROUP], dt.float32, tag="b2col")
        selsT = const.tile([128, NPAIR * 128], dt.bfloat16, tag="sels")
        nc.sync.dma_start(sclT[:], scl_d[:])
        # Split the 2 MiB selector load so the head of the pair loop isn't
        # gated on the whole tile.
        qs = NPAIR * 128 // 4
        for s in range(4):
            nc.sync.dma_start(selsT[:, s * qs : (s + 1) * qs], sels_d[:, s * qs : (s + 1) * qs])
        nc.sync.dma_start(biaT[:], bia_d[:])
        nc.sync.dma_start(bianegT[:], bianeg_d[:])

        # HAM warmup: keep the PE busy through the clock-gate activity window
        # while the x transposes land.  fp32 stationary, nothing reads psw.
        psw = ps_o.tile([128, 2, 512], dt.float32, tag="ps_out")
        for w in range(NWARM):
            nc.tensor.matmul(
                psw[0:NPAIR, 0, 0:NPAIR], sclT[:], sclT[:], start=True, stop=True
            )

        # Transpose x straight from DRAM into SBUF via the DMA xbar.
        xts = []
        for c in range(nchunk):
            xta = xt_p.tile([128, CH], dt.bfloat16, tag="xta")
            xtb = xt_p.tile([128, CH], dt.bfloat16, tag="xtb")
            for hh in range(2):
                r0, r1 = c * CH + hh * (CH // 2), c * CH + (hh + 1) * (CH // 2)
                c0, c1 = hh * (CH // 2), (hh + 1) * (CH // 2)
                nc.scalar.dma_start_transpose(xta[:, c0:c1], x_ila_d[r0:r1, :])
                nc.sync.dma_start_transpose(xtb[:, c0:c1], x_ilb_d[r0:r1, :])
            xts.append((xta, xtb))
            if c == 0:
                nc.sync.dma_start(w2T[:], w2sb_d[:])
                nc.sync.dma_start(b2colT[:], b2col_d[:])

        for c in range(nchunk):
            xta, xtb = xts[c]

            def out_l2(g, hts):
                # L2 matmuls + bias drain straight into [fe, b] layout.
                po = ps_o.tile([128, 2, 512], dt.float32, tag="ps_out")
                for q in range(4):
                    j = 4 * g + q
                    for half in range(2):
                        nc.tensor.matmul(
                            po[32 * q : 32 * q + 32, half, :],
                            w2T[:, 32 * j : 32 * j + 32],
                            hts[q][:, 512 * half : 512 * (half + 1)],
                            start=True,
                            stop=True,
                            tile_position=(0, 32 * q),
                        )
                ot = outsb_p.tile([128, CH], dt.float16, tag="ot")
                nc.vector.tensor_scalar(
                    ot[:], po[:], b2colT[:, g : g + 1], None, ALU.add
                )
                nc.sync.dma_start(
                    out_d[128 * g : 128 * (g + 1), c * CH : (c + 1) * CH], ot[:]
                )

            pend_l2 = None
            for g in range(NGROUP):
                hts = []
                for q in range(4):
                    j = 4 * g + q
                    xt = xta if j < 32 else xtb
                    r0 = _strip_row(j)
                    s0 = 32 * (j % 4)
                    sel = selsT[s0 : s0 + 32, 128 * j : 128 * (j + 1)]
                    mov = xt[s0 : s0 + 32, :]
                    ps = ps_x.tile([128, CH], dt.float32, tag="ps_x")
                    for half in range(2):
                        nc.tensor.matmul(
                            ps[:, 512 * half : 512 * (half + 1)],
                            sel,
                            mov[:, 512 * half : 512 * (half + 1)],
                            start=True,
                            stop=True,
                            tile_position=(s0, 0),
                        )
                    ht = h_p.tile([128, CH], dt.float16, tag="h")
                    if _act_pair(j):
                        nc.scalar.activation(
                            ht[:],
                            ps[:],
                            AF.Relu,
                            bias=biaT[:, j : j + 1],
                            scale=sclT[:, j : j + 1],
                        )
                    else:
                        nc.vector.tensor_scalar(
                            ht[:],
                            ps[:],
                            sclT[:, j : j + 1],
                            bianegT[:, j : j + 1],
                            ALU.mult,
                            ALU.max,
                        )
                    hts.append(ht)
                if pend_l2 is not None:
                    out_l2(*pend_l2)
                pend_l2 = (g, hts)
            out_l2(*pend_l2)

    nc.compile()
    return nc


_NC_CACHE = {}


def _get_program(nrows):
    if nrows not in _NC_CACHE:
        _NC_CACHE[nrows] = _build(nrows)
    return _NC_CACHE[nrows]


def kernel(x, W1, b1, W2, b2, _trace=False):
    from concourse.bass_utils import run_bass_kernel_spmd

    x = np.asarray(x, np.float32)
    cfg = _pack_weights(W1, b1, W2, b2)
    nc = _get_program(BC)
    wkeys = ("scl", "bia", "bianeg", "w2sb", "b2col", "sels")
    in_maps = []
    for c in range(NCORES):
        xa, xb = _prep_x(x[c * BC : (c + 1) * BC])
        m = {"x_ila": xa, "x_ilb": xb}
        for k in wkeys:
            m[k] = cfg[k]
        in_maps.append(m)
    res = run_bass_kernel_spmd(
        nc, in_maps, core_ids=list(range(NCORES)), trace=_trace
    )
    out = np.empty((B, FE), np.float32)
    for c in range(NCORES):
        oc = np.asarray(res.results[c]["out"])          # [FE, BC] fp16
        out[c * BC : (c + 1) * BC] = oc.T.astype(np.float32)
    if _trace:
        kernel.last_result = res
    return out
